# revision 31
# baseline (speedup 1.0000x reference)
"""Bass/Trainium2 kernel for nn_BridgeNodes: per-group thresholded sigmoid
similarity map  out[g] = where(sigmoid(nodes_g @ nodes_g.T) < 0.6, 0, sigmoid(...)).

The map is exactly symmetric (dot(i,j) and dot(j,i) accumulate in the same
order on the PE), so only upper-triangle tiles are computed on device; the
host mirrors the lower triangle during unshard.

Sharding: 8 cores = (group, row-parity). Core i handles group i//2 and the
16 row-blocks m = 2k + (i%2) (k=0..15, 128 rows each) of that group. For
row-block m only column chunks j >= floor(m/4) (512 cols each) are computed
— chunk counts per k are parity-independent, so one SPMD program serves all
cores; the host supplies each core's row-blocks gathered into rows_t.

Per-chunk pipeline:
  PE    : matmul [K=128, M=128, N=512] -> PSUM  (dot = x, native fp32)
  ACT   : s = Sigmoid(psum)            -> SBUF
  DVE   : out = (psum >= c) * s        -> SBUF   (one fused
          scalar_tensor_tensor: op0=is_ge vs c, op1=mult by s;
          mask decided on the raw fp32 dot, exact 0.0 for dropped)
  DMA   : one store per row-block of the computed column suffix
"""

import numpy as np

import concourse.bacc as bacc
import concourse.bass as bass
import concourse.mybir as mybir
import concourse.tile as tile
from concourse.bass_utils import run_bass_kernel_spmd

G = 4          # groups
N = 4096       # nodes per group
F = 128        # feature dim
CORES = 8
MT = 128       # rows per m-tile (PSUM partition dim)
NB = N // MT   # 32 row-blocks per group
KT = NB // 2   # 16 row-blocks per core
R = KT * MT    # 2048 rows handled per core
CW = 512       # columns per chunk (one PSUM bank of fp32)

# Decision boundary in dot space: smallest fp32 x with sigmoid(x) >= f32(0.6).
# fp64-exact boundary is f32(ln 1.5) + 4 ulp = 0x3ecf9923.
THRESH_C = float(np.frombuffer(np.uint32(0x3ECF9923).tobytes(), np.float32)[0])


def _c0(k):
    # first computed column for local row-block k: the diagonal of global
    # row-block m = 2k+p starts at m*128; 2k*128 = k*256 covers both
    # parities (p=1 recomputes 128 sub-diagonal cols, overwritten by the
    # host mirror)
    return k * 2 * MT


def _w(k):
    # computed width (cols) for local row-block k
    return N - _c0(k)


_OFF = np.concatenate([[0], np.cumsum([_w(k) for k in range(KT)])]).astype(int)
TOTW = int(_OFF[-1])  # 34816 — packed output cols

_NC_CACHE = {}


def _j0(k):
    # first computed 512-col chunk for local row-block k (global m = 2k+p;
    # floor((2k+p)/4) is parity-independent)
    return (2 * k) // 4


def _build_nc():
    if "nc" in _NC_CACHE:
        return _NC_CACHE["nc"]
    f32 = mybir.dt.float32
    nc = bacc.Bacc()
    rows_t = nc.dram_tensor("rows_t", [F, R], f32, kind="ExternalInput")
    cols_t = nc.dram_tensor("cols_t", [F, N], f32, kind="ExternalInput")
    out = nc.dram_tensor("out", [MT, TOTW], f32, kind="ExternalOutput")

    with tile.TileContext(nc) as tc:
        with (
            tc.tile_pool(name="inp", bufs=1) as inp,
            tc.tile_pool(name="ps", bufs=8, space="PSUM") as psp,
            tc.tile_pool(name="sig", bufs=3) as sigp,
            tc.tile_pool(name="res", bufs=3) as resp,
        ):
            rt = inp.tile([F, R], f32)
            ct = inp.tile([F, N], f32)
            # split loads so the first matmuls start as soon as their
            # slices land instead of waiting for the full 3 MiB; each
            # dma_start costs ~0.6us of serial HWDGE dispatch, so keep
            # the piece count low
            nc.sync.dma_start(ct[:, :CW], cols_t[:, :CW])
            nc.sync.dma_start(rt[:, :MT], rows_t[:, :MT])
            nc.sync.dma_start(ct[:, CW : 3 * CW], cols_t[:, CW : 3 * CW])
            nc.sync.dma_start(ct[:, 3 * CW : 5 * CW], cols_t[:, 3 * CW : 5 * CW])
            nc.sync.dma_start(ct[:, 5 * CW :], cols_t[:, 5 * CW :])
            nc.sync.dma_start(rt[:, MT:], rows_t[:, MT:])

            # prime the PE's activity monitor while inputs stream in: tiny
            # matmuls on a memset tile (no DMA dependency) keep the clock
            # gate ramping so the first real matmuls run warm
            wsrc = inp.tile([MT, 64], f32)
            nc.vector.memset(wsrc[:], 0.0)
            warm = psp.tile([MT, CW], f32, tag="ps")
            for _ in range(8):
                nc.tensor.matmul(warm[:64, :64], wsrc[:, :64], wsrc[:, :64])

            for k in range(KT):
                ncols = _w(k)
                s = sigp.tile([MT, ncols], f32, tag="sig")
                o = resp.tile([MT, ncols], f32, tag="res")
                for c in range(0, ncols, CW):
                    cw = min(CW, ncols - c)
                    col = _c0(k) + c
                    ps = psp.tile([MT, CW], f32)
                    nc.tensor.matmul(
                        ps[:, :cw],
                        rt[:, k * MT : (k + 1) * MT],
                        ct[:, col : col + cw],
                    )
                    sq = s[:, c : c + cw]
                    nc.scalar.activation(
                        sq, ps[:, :cw], mybir.ActivationFunctionType.Sigmoid
                    )
                    nc.vector.scalar_tensor_tensor(
                        o[:, c : c + cw],
                        ps[:, :cw],
                        THRESH_C,
                        sq,
                        op0=mybir.AluOpType.is_ge,
                        op1=mybir.AluOpType.mult,
                    )
                nc.sync.dma_start(out[:, _OFF[k] : _OFF[k + 1]], o[:])
    nc.finalize()
    _NC_CACHE["nc"] = nc
    return nc


def _in_maps(nodes):
    maps = []
    for core in range(CORES):
        g, p = core // 2, core % 2
        ct = np.ascontiguousarray(nodes[g].T)  # [F, N]
        # gather this core's row-blocks: m = 2k+p
        rt = np.ascontiguousarray(
            ct.reshape(F, NB, MT)[:, p::2, :].reshape(F, R)
        )
        maps.append({"rows_t": rt, "cols_t": ct})
    return maps


def _assemble(results):
    full = np.zeros((G, N, N), np.float32)
    for core in range(CORES):
        g, p = core // 2, core % 2
        packed = results[core]["out"]
        for k in range(KT):
            m = 2 * k + p
            full[g, m * MT : (m + 1) * MT, _c0(k):] = packed[:, _OFF[k] : _OFF[k + 1]]
    # mirror strictly-lower row-blocks from the computed upper triangle
    for g in range(G):
        x = full[g]
        for bi in range(NB):
            for bj in range(bi):
                x[bi * MT : (bi + 1) * MT, bj * MT : (bj + 1) * MT] = x[
                    bj * MT : (bj + 1) * MT, bi * MT : (bi + 1) * MT
                ].T
    return full


def kernel(nodes):
    nodes = np.ascontiguousarray(np.asarray(nodes, dtype=np.float32))
    assert nodes.shape == (G, N, F), nodes.shape
    nc = _build_nc()
    res = run_bass_kernel_spmd(nc, _in_maps(nodes), list(range(CORES))).results
    return _assemble(res)
